# revision 3
# baseline (speedup 1.0000x reference)
"""Trainium2 Bass kernel for nn_Diag: out = x * exp(betas), broadcast over (B, C).

Full shapes: x_real/x_imag (32, 8, 256, 256) f32, betas (65536,) f32.
Sharding: pure data parallel on batch across 8 cores -> per-core (4, 8, 256, 256)
viewed as (32, 65536). betas replicated.

Per-core kernel layout: hw index j = p*512 + f with p in [0,128) partitions,
f in [0,512). 16 images per SBUF tile -> [128, 8192] f32 (4 MiB) tiles, scale
tile exp(betas) replicated 16x along the free dim so each tile is a single DVE
tensor_mul. Loads issued on the SP HWDGE ring, stores on the ACT HWDGE ring.
"""

import numpy as np

import concourse.bacc as bacc
import concourse.mybir as mybir
import concourse.tile as tile
from concourse import bass_utils

B, C, H, W = 32, 8, 256, 256
DIM = H * W  # 65536
N_CORES = 8
B_LOC = B // N_CORES  # 4 batches per core
N_IMG = B_LOC * C  # 32 images per core per tensor
P = 128
F = DIM // P  # 512
K = 16  # images per SBUF tile
G = N_IMG // K  # tile groups per tensor

_NC_CACHE = {}


def _build(n_iters=1):
    f32 = mybir.dt.float32
    nc = bacc.Bacc("TRN2", target_bir_lowering=False, debug=False)

    xr = nc.dram_tensor("x_real", (N_IMG, DIM), f32, kind="ExternalInput").ap()
    xi = nc.dram_tensor("x_imag", (N_IMG, DIM), f32, kind="ExternalInput").ap()
    bt = nc.dram_tensor("betas", (DIM,), f32, kind="ExternalInput").ap()
    our = nc.dram_tensor("out_real", (N_IMG, DIM), f32, kind="ExternalOutput").ap()
    oui = nc.dram_tensor("out_imag", (N_IMG, DIM), f32, kind="ExternalOutput").ap()

    with tile.TileContext(nc) as tc:
        with (
            tc.tile_pool(name="scale", bufs=1) as scale_pool,
            tc.tile_pool(name="io", bufs=4) as io_pool,
        ):

            def body(_i=None):
                beta_t = scale_pool.tile([P, F], f32)
                nc.sync.dma_start(beta_t[:], bt.rearrange("(p f) -> p f", p=P))

                scale = scale_pool.tile([P, K * F], f32)
                nc.scalar.activation(
                    scale[:, 0:F], beta_t[:], mybir.ActivationFunctionType.Exp
                )
                # log-doubling replication of exp(betas) along the free dim
                width = F
                while width < K * F:
                    w = min(width, K * F - width)
                    nc.vector.tensor_copy(scale[:, width : width + w], scale[:, 0:w])
                    width += w

                for src, dst in ((xr, our), (xi, oui)):
                    sv = src.rearrange("(g k) (p f) -> g p k f", k=K, p=P)
                    dv = dst.rearrange("(g k) (p f) -> g p k f", k=K, p=P)
                    for g in range(G):
                        t = io_pool.tile([P, K * F], f32, tag="io")
                        tv = t[:].rearrange("p (k f) -> p k f", f=F)
                        nc.sync.dma_start(tv, sv[g])
                        nc.vector.tensor_mul(t[:], t[:], scale[:])
                        nc.scalar.dma_start(dv[g], tv)

            if n_iters == 1:
                body()
            else:
                with tc.For_i(0, n_iters, 1) as i:
                    body(i)

    nc.compile()
    return nc


def _get_nc(n_iters=1):
    if n_iters not in _NC_CACHE:
        _NC_CACHE[n_iters] = _build(n_iters)
    return _NC_CACHE[n_iters]


def _shard(x: np.ndarray) -> list[np.ndarray]:
    x2 = np.ascontiguousarray(x).reshape(B * C, DIM)
    per = B_LOC * C
    return [x2[i * per : (i + 1) * per] for i in range(N_CORES)]


def run_cores(x_real, x_imag, betas, trace=False, n_iters=1, **kw):
    nc = _get_nc(n_iters)
    xr_s = _shard(x_real)
    xi_s = _shard(x_imag)
    betas = np.ascontiguousarray(betas, dtype=np.float32)
    in_maps = [
        {"x_real": xr_s[i], "x_imag": xi_s[i], "betas": betas} for i in range(N_CORES)
    ]
    res = bass_utils.run_bass_kernel_spmd(
        nc, in_maps, core_ids=list(range(N_CORES)), trace=trace, **kw
    )
    out_r = np.concatenate([r["out_real"] for r in res.results], axis=0)
    out_i = np.concatenate([r["out_imag"] for r in res.results], axis=0)
    out_r = out_r.reshape(B, C, H, W)
    out_i = out_i.reshape(B, C, H, W)
    return (out_r, out_i), res


def kernel(x_real, x_imag, betas):
    (out_r, out_i), _ = run_cores(x_real, x_imag, betas)
    return out_r, out_i


# revision 7
# speedup vs baseline: 1.2660x; 1.2660x over previous
"""Trainium2 Bass kernel for nn_Diag: out = x * exp(betas), broadcast over (B, C).

Full shapes: x_real/x_imag (32, 8, 256, 256) f32, betas (65536,) f32.
Sharding: pure data parallel on batch across 8 cores -> per-core (4, 8, 256, 256)
viewed as (32, 65536). betas replicated.

Per-core kernel layout: hw index j = p*512 + f with p in [0,128) partitions,
f in [0,512). 16 images per SBUF tile -> [128, 8192] f32 (4 MiB) tiles, scale
tile exp(betas) replicated 16x along the free dim so each tile is a single DVE
tensor_mul. Loads issued on the SP HWDGE ring, stores on the ACT HWDGE ring.
"""

import numpy as np

import concourse.bacc as bacc
import concourse.mybir as mybir
import concourse.tile as tile
from concourse import bass_utils

B, C, H, W = 32, 8, 256, 256
DIM = H * W  # 65536
N_CORES = 8
B_LOC = B // N_CORES  # 4 batches per core
N_IMG = B_LOC * C  # 32 images per core per tensor
P = 128
F = DIM // P  # 512
K = 16  # images per SBUF tile
G = N_IMG // K  # tile groups per tensor

_NC_CACHE = {}


def _build(n_iters=1, k=K, bufs=4, mul=True, ring_mode="split", mul_split=1, gp_frac=0):
    """ring_mode: 'split' = loads on SP ring, stores on ACT ring;
    'swap' = the reverse; 'alt' = alternate per tile group.
    mul_split: issue the per-tile multiply (and its store) in this many
    free-dim chunks so stores start before the whole tile is multiplied.
    gp_frac: out of every 4 mul-chunks, how many go to GPSIMD instead of DVE."""
    f32 = mybir.dt.float32
    g_per = N_IMG // k
    nc = bacc.Bacc("TRN2", target_bir_lowering=False, debug=False)

    xr = nc.dram_tensor("x_real", (N_IMG, DIM), f32, kind="ExternalInput").ap()
    xi = nc.dram_tensor("x_imag", (N_IMG, DIM), f32, kind="ExternalInput").ap()
    bt = nc.dram_tensor("betas", (DIM,), f32, kind="ExternalInput").ap()
    our = nc.dram_tensor("out_real", (N_IMG, DIM), f32, kind="ExternalOutput").ap()
    oui = nc.dram_tensor("out_imag", (N_IMG, DIM), f32, kind="ExternalOutput").ap()

    with tile.TileContext(nc) as tc:
        with (
            tc.tile_pool(name="scale", bufs=1) as scale_pool,
            tc.tile_pool(name="io", bufs=bufs) as io_pool,
        ):

            def body(_i=None):
                beta_t = scale_pool.tile([P, F], f32)
                nc.sync.dma_start(beta_t[:], bt.rearrange("(p f) -> p f", p=P))

                scale = scale_pool.tile([P, k * F], f32)
                nc.scalar.activation(
                    scale[:, 0:F], beta_t[:], mybir.ActivationFunctionType.Exp
                )
                # log-doubling replication of exp(betas) along the free dim
                width = F
                while width < k * F:
                    w = min(width, k * F - width)
                    nc.vector.tensor_copy(scale[:, width : width + w], scale[:, 0:w])
                    width += w

                n = 0
                for src, dst in ((xr, our), (xi, oui)):
                    sv = src.rearrange("(g kk) (p f) -> g p kk f", kk=k, p=P)
                    dv = dst.rearrange("(g kk) (p f) -> g p kk f", kk=k, p=P)
                    for g in range(g_per):
                        if ring_mode == "split":
                            ld, st = nc.sync, nc.scalar
                        elif ring_mode == "swap":
                            ld, st = nc.scalar, nc.sync
                        else:
                            ld, st = (
                                (nc.sync, nc.scalar)
                                if n % 2 == 0
                                else (nc.scalar, nc.sync)
                            )
                        n += 1
                        t = io_pool.tile([P, k * F], f32, tag="io")
                        tv = t[:].rearrange("p (kk f) -> p kk f", f=F)
                        ld.dma_start(tv, sv[g])
                        if mul and mul_split == 1:
                            nc.vector.tensor_mul(t[:], t[:], scale[:])
                            st.dma_start(dv[g], tv)
                        else:
                            kc = k // mul_split
                            for m in range(mul_split):
                                tslice = t[:, m * kc * F : (m + 1) * kc * F]
                                if mul:
                                    eng = (
                                        nc.gpsimd
                                        if (n * mul_split + m) % 4 < gp_frac
                                        else nc.vector
                                    )
                                    eng.tensor_mul(
                                        tslice,
                                        tslice,
                                        scale[:, m * kc * F : (m + 1) * kc * F],
                                    )
                                st.dma_start(
                                    dv[g, :, m * kc : (m + 1) * kc, :],
                                    tv[:, m * kc : (m + 1) * kc, :],
                                )

            if n_iters == 1:
                body()
            else:
                with tc.For_i(0, n_iters, 1) as i:
                    body(i)

    nc.compile()
    return nc


def _get_nc(n_iters=1, **kw):
    key = (n_iters, tuple(sorted(kw.items())))
    if key not in _NC_CACHE:
        _NC_CACHE[key] = _build(n_iters, **kw)
    return _NC_CACHE[key]


def _shard(x: np.ndarray) -> list[np.ndarray]:
    x2 = np.ascontiguousarray(x, dtype=np.float32).reshape(B * C, DIM)
    per = B_LOC * C
    return [x2[i * per : (i + 1) * per] for i in range(N_CORES)]


def run_cores(x_real, x_imag, betas, trace=False, n_iters=1, **kw):
    nc = _get_nc(n_iters)
    xr_s = _shard(x_real)
    xi_s = _shard(x_imag)
    betas = np.ascontiguousarray(betas, dtype=np.float32)
    in_maps = [
        {"x_real": xr_s[i], "x_imag": xi_s[i], "betas": betas} for i in range(N_CORES)
    ]
    res = bass_utils.run_bass_kernel_spmd(
        nc, in_maps, core_ids=list(range(N_CORES)), trace=trace, **kw
    )
    out_r = np.concatenate([r["out_real"] for r in res.results], axis=0)
    out_i = np.concatenate([r["out_imag"] for r in res.results], axis=0)
    out_r = out_r.reshape(B, C, H, W)
    out_i = out_i.reshape(B, C, H, W)
    return (out_r, out_i), res


def kernel(x_real, x_imag, betas):
    (out_r, out_i), _ = run_cores(x_real, x_imag, betas)
    return out_r, out_i
